# revision 5
# baseline (speedup 1.0000x reference)
"""Trainium2 Bass kernel for nn_CliffordEngine (8-core data-parallel over batch).

Model: 4 passes of (grouped causal 3x3x3 conv over 32^3 grid -> tanh ->
8x8 cross-field mix -> alpha blend), then a sigmoid gate vs the original
state.  B=16, F=8 fields, 8 multivector components, G=32.

Mapping: each core takes 2 batch elements.  SBUF layout: partitions =
(b2, f8, c8) = 128; free dim = causally padded 34^3 volume stored in
fp8e4 (state scaled by S_X).  The conv runs as fp8 DoubleRow matmuls:
each instruction contracts TWO taps at once (K=256) using a strided
3D access pattern [128, 2(pair-delta), N] over the padded volume.

Scheduling: software-pipelined.  Conv matmuls for a round of `ROUNDS`
subtiles are emitted tap-major (consecutive matmuls share lhsT so the
PE can skip stationary reloads); the mix matmuls of the previous round
are emitted after the next round's convs so the PE never stalls on the
Act-engine tanh.  The final gate is computed with DVE+Pool ops split so
no single engine exceeds the PE's per-subtile budget.
"""

import numpy as np

import concourse.bacc as bacc
import concourse.tile as tile
from concourse import mybir
from concourse.ap import AP
from concourse.bass_utils import run_bass_kernel_spmd

# Cl(3,0) Cayley table, transcribed from the reference model.
_TABLE = [
    [(0, 0, 1), (1, 1, 1), (2, 2, 1), (3, 3, 1), (4, 4, -1), (5, 5, -1), (6, 6, -1), (7, 7, -1)],
    [(1, 0, 1), (0, 1, 1), (4, 2, -1), (5, 3, -1), (2, 4, 1), (3, 5, 1), (7, 6, -1), (6, 7, -1)],
    [(2, 0, 1), (4, 1, 1), (0, 2, 1), (6, 3, -1), (1, 4, -1), (7, 5, 1), (3, 6, 1), (5, 7, 1)],
    [(3, 0, 1), (5, 1, 1), (6, 2, 1), (0, 3, 1), (7, 4, -1), (1, 5, -1), (2, 6, -1), (4, 7, -1)],
    [(4, 0, 1), (2, 1, 1), (1, 2, -1), (7, 3, 1), (0, 4, 1), (6, 5, -1), (5, 6, 1), (3, 7, 1)],
    [(5, 0, 1), (3, 1, 1), (7, 2, -1), (1, 3, -1), (6, 4, 1), (0, 5, 1), (4, 6, -1), (2, 7, -1)],
    [(6, 0, 1), (7, 1, 1), (3, 2, 1), (2, 3, -1), (5, 4, -1), (4, 5, 1), (0, 6, 1), (1, 7, 1)],
    [(7, 0, 1), (6, 1, 1), (5, 2, -1), (4, 3, 1), (3, 4, 1), (2, 5, -1), (1, 6, 1), (0, 7, 1)],
]

B, F, C, G = 16, 8, 8, 32
P_PASSES = 4
NCORES = 8
BL = B // NCORES            # local batch per core = 2
NPART = BL * F * C          # 128
GP = G + 2                  # padded extent 34
G3 = G * G * G
NTAPS = 27
ROWST = GP                  # 34
PLANEST = GP * GP           # 1156
PITCH = GP * GP * GP + 128  # padded volume + tail slop for pair reads
S_X = 16.0                  # fp8 state scale
S_W = 128.0                 # fp8 conv weight scale
HEIGHTS = [(0, 16), (16, 16)]   # (y0, rows) subtiles per plane
TAPS = [(kd, kh, kw) for kd in range(3) for kh in range(3) for kw in range(3)]

F32 = mybir.dt.float32
BF16 = mybir.dt.bfloat16
FP8 = mybir.dt.float8e4
NP_FP8 = mybir.dt.np(FP8)
NP_BF16 = mybir.dt.np(BF16)


def _softmax(x, axis):
    m = np.max(x, axis=axis, keepdims=True)
    e = np.exp(x - m)
    return e / np.sum(e, axis=axis, keepdims=True)


def _host_params(all_weights, all_biases, field_mix_logits, pass_alpha_logit,
                 gate_weight, gate_bias):
    """Precompute device weight tensors (runtime data, not baked into the NEFF)."""
    T = np.zeros((8, 8, 8), np.float32)
    for i, row in enumerate(_TABLE):
        for j, k, s in row:
            T[i, j, k] = s
    aw = np.asarray(all_weights, np.float32)          # [F, P, 27, 8]
    # W_eff[f,p,i,j,t] : out component i, in component j, tap t
    W = np.einsum('ijk,fpck->fpijc', T, aw)            # [F,P,8,8,27]
    mix = _softmax(np.asarray(field_mix_logits, np.float32), axis=2)  # [P,F,F] (g,f)
    alpha = 1.0 / (1.0 + np.exp(-np.asarray(pass_alpha_logit, np.float32)))  # [P]

    # conv lhsT: [k=(b,f,j), p, t, m=(b,f,i)] block-diagonal over (b, f), fp8
    convw = np.zeros((BL, F, C, P_PASSES, NTAPS, BL, F, C), np.float32)
    A = np.transpose(W, (0, 3, 1, 4, 2))               # [f, j, p, t, i]
    for b in range(BL):
        for f in range(F):
            convw[b, f, :, :, :, b, f, :] = A[f]
    convw = convw.reshape(NPART, P_PASSES, NTAPS, NPART)
    convw8 = np.clip(convw * S_W, -240.0, 240.0).astype(NP_FP8)

    # mix lhsT: [k=(b,f,ci), p, m=(b,g,co)] = S_X (1-alpha_p) mix[p,g,f] delta_b delta_c
    mixw = np.zeros((BL, F, C, P_PASSES, BL, F, C), np.float32)
    for b in range(BL):
        for p in range(P_PASSES):
            M2 = S_X * (1.0 - alpha[p]) * mix[p].T     # [f, g]
            for ci in range(C):
                mixw[b, :, ci, p, b, :, ci] = M2
    mixw = mixw.reshape(NPART, P_PASSES, NPART).astype(NP_BF16)

    # per-partition scalars: cols 0-3 bias_p, 4-7 alpha_p, 8 = -gw, 9 = -gb,
    # 10 = alpha3/S_X, 11+p = S_W * diag of the folded center tap of pass p
    pvec = np.zeros((BL, F, C, 16), np.float32)
    pvec = pvec.reshape(NPART, 16)
    ab = np.asarray(all_biases, np.float32).reshape(F, P_PASSES, C)
    pv4 = pvec.reshape(BL, F, C, 16)
    for p in range(P_PASSES):
        pv4[:, :, :, p] = ab[None, :, p, :]
        pv4[:, :, :, 4 + p] = alpha[p]
    pv4[:, :, :, 8] = -np.asarray(gate_weight, np.float32)[None]
    pv4[:, :, :, 9] = -np.asarray(gate_bias, np.float32)[None]
    pv4[:, :, :, 10] = alpha[P_PASSES - 1] / S_X
    # last pass's mix output feeds the fused gate directly (f32, unscaled)
    mixw[:, P_PASSES - 1] = (mixw[:, P_PASSES - 1].astype(np.float32)
                             / S_X).astype(NP_BF16)

    # active taps per pass; fold one diagonal tap to DVE when the count is odd
    active, fold = [], []
    for p in range(P_PASSES):
        nz = [t for t in range(NTAPS) if np.any(convw[:, p, t, :] != 0)]
        if not nz:
            nz = [0]
        ft = None
        if len(nz) % 2 == 1 and len(nz) > 1:
            for t in reversed(nz):
                blk = convw[:, p, t, :]
                if not np.any(blk[~np.eye(NPART, dtype=bool)]):
                    ft = t
                    break
            if ft is not None:
                nz.remove(ft)
                pvec[:, 11 + p] = S_W * np.diag(convw[:, p, ft, :])
        active.append(tuple(nz))
        fold.append(ft)
    return convw8, mixw, pvec, tuple(active), tuple(fold)


def build_nc(repeat_passes=1, active_taps=None, fold_taps=None,
             strided_rhs=True, pipeline=True, rounds=2):
    """Build the per-core Bass program.

    pipeline: defer each round's mix matmuls until after the next round's
    conv matmuls so the PE never waits on the Act-engine tanh.
    rounds: subtiles per round; conv matmuls are emitted tap-major across
    the round so consecutive matmuls share the same stationary weights.
    """
    if active_taps is None:
        active_taps = tuple(tuple(range(NTAPS)) for _ in range(P_PASSES))
    if fold_taps is None:
        fold_taps = (None,) * P_PASSES
    nc = bacc.Bacc("TRN2", target_bir_lowering=False, debug=False,
                   num_devices=NCORES)
    state_in = nc.dram_tensor("state_in", [NPART, G3], F32, kind="ExternalInput")
    convw_d = nc.dram_tensor("convw", [NPART, P_PASSES, NTAPS, NPART], FP8,
                             kind="ExternalInput")
    mixw_d = nc.dram_tensor("mixw", [NPART, P_PASSES, NPART], BF16,
                            kind="ExternalInput")
    pvec_d = nc.dram_tensor("pvec", [NPART, 16], F32, kind="ExternalInput")
    out_d = nc.dram_tensor("out", [NPART, G3], F32, kind="ExternalOutput")

    # Persistent padded state buffers (ping/pong), fp8, flat with tail slop.
    ping = nc.alloc_sbuf_tensor("ping", [NPART, PITCH], FP8)
    pong = nc.alloc_sbuf_tensor("pong", [NPART, PITCH], FP8)

    def vol_ap(vol, off, dims):
        return AP(vol, off, [[PITCH, NPART]] + dims)

    # per-subtile metadata: (pass, z0, y0, hh, cur, nxt, global index)
    subtiles = []
    vols = [ping, pong]
    for rp in range(repeat_passes):
        for p in range(P_PASSES):
            gp_idx = rp * P_PASSES + p
            cur, nxt = vols[gp_idx % 2], vols[(gp_idx + 1) % 2]
            last_pass = (rp == repeat_passes - 1 and p == P_PASSES - 1)
            for z0 in range(G):
                for (y0, hh) in HEIGHTS:
                    subtiles.append(dict(p=p, z0=z0, y0=y0, hh=hh, cur=cur,
                                         nxt=nxt, last=last_pass))
    # group into rounds (same pass within a round; pass lengths are
    # multiples of `rounds` when rounds divides 64)
    assert 64 % rounds == 0
    round_list = [subtiles[i:i + rounds] for i in range(0, len(subtiles),
                                                        rounds)]

    with tile.TileContext(nc) as tc:
        with (
            tc.tile_pool(name="const", bufs=1) as constp,
            tc.tile_pool(name="stage", bufs=3) as stagep,
            tc.tile_pool(name="ytile", bufs=2 * rounds + 1) as yp,
            tc.tile_pool(name="gtile", bufs=3) as gp_,
            tc.tile_pool(name="psum1", bufs=2 * rounds, space="PSUM") as ps1p,
            tc.tile_pool(name="psum2", bufs=2 * rounds, space="PSUM") as ps2p,
        ):
            # memset only the halo/pad regions (interior is fully overwritten
            # by the init load / blends).
            for vol in (ping, pong):
                nc.gpsimd.memset(vol[:, 0:2 * PLANEST + 2 * ROWST + 2], 0)
                nc.gpsimd.memset(
                    AP(vol, 3 * PLANEST, [[PITCH, NPART], [PLANEST, GP - 3],
                                          [1, 2 * ROWST + 2]]), 0)
                nc.gpsimd.memset(
                    AP(vol, 2 * PLANEST + 2 * ROWST,
                       [[PITCH, NPART], [PLANEST, GP - 2], [ROWST, G], [1, 2]]),
                    0)
                nc.gpsimd.memset(vol[:, GP * PLANEST:], 0)

            convw_sb = constp.tile([NPART, P_PASSES, NTAPS, NPART], FP8,
                                   tag="convw")
            for p in range(P_PASSES):  # split so pass 0 weights land first
                nc.sync.dma_start(convw_sb[:, p], convw_d[:, p])
            mixw_sb = constp.tile([NPART, P_PASSES, NPART], BF16, tag="mixw")
            nc.sync.dma_start(mixw_sb[:], mixw_d[:])
            pvec_sb = constp.tile([NPART, 16], F32, tag="pvec")
            nc.sync.dma_start(pvec_sb[:], pvec_d[:])

            # initial load: f32 planes -> fp8 (x S_X) padded interior
            for z in range(G):
                st = stagep.tile([NPART, G * G], F32, tag="stage")
                nc.sync.dma_start(st[:], state_in[:, z * 1024:(z + 1) * 1024])
                nc.scalar.activation(
                    out=vol_ap(ping, (z + 2) * PLANEST + 2 * ROWST + 2,
                               [[ROWST, G], [1, G]]),
                    in_=st[:].rearrange("p (a b) -> p a b", a=G),
                    func=mybir.ActivationFunctionType.Copy, scale=S_X)

            def sub_ap(base_ap, off, dims):
                return AP(base_ap.tensor, base_ap.offset + off,
                          [list(base_ap.ap[0])] + dims)

            def conv_lhsT(p, tA, tB):
                off = (p * NTAPS + tA) * NPART
                return sub_ap(convw_sb[:], off,
                              [[(tB - tA) * NPART, 2], [1, NPART]])

            def emit_convs(rnd, tiles_out):
                """Emit the conv matmuls for one round, tap-major."""
                p = rnd[0]["p"]
                act = active_taps[p]
                pairs = [(act[2 * i], act[2 * i + 1])
                         for i in range(len(act) // 2)]
                single = act[-1] if len(act) % 2 else None
                ngroups = len(pairs) + (1 if single is not None else 0)
                for s in rnd:
                    if strided_rhs:
                        ps1 = ps1p.tile([NPART, s["hh"], G], F32,
                                        space="PSUM", tag="ps1", name="ps1")
                    else:
                        ps1 = ps1p.tile([NPART, s["hh"] * ROWST], F32,
                                        space="PSUM", tag="ps1", name="ps1")
                    tiles_out.append(ps1)
                gi = 0
                for (tA, tB) in pairs:
                    kdA, khA, kwA = TAPS[tA]
                    kdB, khB, kwB = TAPS[tB]
                    for si, s in enumerate(rnd):
                        offA = ((s["z0"] + kdA) * PLANEST
                                + (s["y0"] + khA) * ROWST + kwA)
                        offB = ((s["z0"] + kdB) * PLANEST
                                + (s["y0"] + khB) * ROWST + kwB)
                        if strided_rhs:
                            rhs = vol_ap(s["cur"], offA,
                                         [[offB - offA, 2],
                                          [ROWST, s["hh"]], [1, G]])
                        else:
                            rhs = vol_ap(s["cur"], offA,
                                         [[offB - offA, 2],
                                          [1, s["hh"] * ROWST]])
                        nc.tensor.matmul(
                            out=tiles_out[-len(rnd) + si][:],
                            lhsT=conv_lhsT(p, tA, tB),
                            rhs=rhs, start=(gi == 0),
                            stop=(gi == ngroups - 1),
                            perf_mode=mybir.MatmulPerfMode.DoubleRow)
                    gi += 1
                if single is not None:
                    kd, kh, kw = TAPS[single]
                    for si, s in enumerate(rnd):
                        off = ((s["z0"] + kd) * PLANEST
                               + (s["y0"] + kh) * ROWST + kw)
                        if strided_rhs:
                            rhs = vol_ap(s["cur"], off,
                                         [[ROWST, s["hh"]], [1, G]])
                        else:
                            rhs = vol_ap(s["cur"], off,
                                         [[1, s["hh"] * ROWST]])
                        nc.tensor.matmul(
                            out=tiles_out[-len(rnd) + si][:],
                            lhsT=convw_sb[:, p, single], rhs=rhs,
                            start=(gi == 0), stop=(gi == ngroups - 1))
                    gi += 1

            def emit_tanh(s, ps1, ys_out):
                """Act tanh (+ optional DVE fold of a diagonal center tap)."""
                p, hh = s["p"], s["hh"]
                if strided_rhs:
                    ps1v = ps1[:]
                else:
                    ps1v = sub_ap(ps1[:], 0, [[ROWST, hh], [1, G]])
                tanh_in = ps1v
                if fold_taps[p] is not None:
                    kd, kh, kw = TAPS[fold_taps[p]]
                    coff = ((s["z0"] + kd) * PLANEST
                            + (s["y0"] + kh) * ROWST + kw)
                    tmp = yp.tile([NPART, hh, G], F32, tag="tmp", name="tmp")
                    nc.vector.scalar_tensor_tensor(
                        out=tmp[:],
                        in0=vol_ap(s["cur"], coff, [[ROWST, hh], [1, G]]),
                        scalar=pvec_sb[:, 11 + p:12 + p],
                        in1=ps1v,
                        op0=mybir.AluOpType.mult,
                        op1=mybir.AluOpType.add,
                    )
                    tanh_in = tmp[:]
                y = yp.tile([NPART, hh, G], BF16, tag="y", name="y")
                nc.scalar.activation(
                    out=y[:], in_=tanh_in,
                    func=mybir.ActivationFunctionType.Tanh,
                    bias=pvec_sb[:, p:p + 1], scale=1.0 / (S_X * S_W),
                )
                ys_out.append(y)

            def emit_mix(s, y, ps2_out):
                p, hh = s["p"], s["hh"]
                ps2 = ps2p.tile([NPART, hh, G], F32, space="PSUM", tag="ps2",
                                name="ps2")
                nc.tensor.matmul(out=ps2[:], lhsT=mixw_sb[:, p], rhs=y[:],
                                 start=True, stop=True)
                ps2_out.append(ps2)

            def emit_blend(s, ps2, old=None):
                p, hh = s["p"], s["hh"]
                z0, y0 = s["z0"], s["y0"]
                intr = (z0 + 2) * PLANEST + (y0 + 2) * ROWST + 2
                if not s["last"]:
                    # nxt = alpha * cur + ps2  (S_X units, fp8)
                    nc.vector.scalar_tensor_tensor(
                        out=vol_ap(s["nxt"], intr, [[ROWST, hh], [1, G]]),
                        in0=vol_ap(s["cur"], intr, [[ROWST, hh], [1, G]]),
                        scalar=pvec_sb[:, 4 + p:5 + p],
                        in1=ps2[:],
                        op0=mybir.AluOpType.mult,
                        op1=mybir.AluOpType.add,
                    )
                    return
                # fused gate (last pass): ps2 here is unscaled, so
                # x4 = (alpha3/S_X) * cur + ps2;
                # out = old + sigmoid(-(gw*old+gb)) * (x4 - old)
                sl = slice(z0 * 1024 + y0 * G, z0 * 1024 + (y0 + hh) * G)
                hg = gp_.tile([NPART, hh, G], F32, tag="hg", name="hg")
                nc.scalar.activation(
                    out=hg[:], in_=old[:],
                    func=mybir.ActivationFunctionType.Sigmoid,
                    bias=pvec_sb[:, 9:10], scale=pvec_sb[:, 8:9],
                )
                x4 = gp_.tile([NPART, hh, G], F32, tag="x4", name="x4")
                nc.vector.scalar_tensor_tensor(
                    out=x4[:],
                    in0=vol_ap(s["cur"], intr, [[ROWST, hh], [1, G]]),
                    scalar=pvec_sb[:, 10:11],
                    in1=ps2[:],
                    op0=mybir.AluOpType.mult,
                    op1=mybir.AluOpType.add,
                )
                d = gp_.tile([NPART, hh, G], F32, tag="d", name="d")
                nc.gpsimd.tensor_tensor(
                    out=d[:], in0=x4[:], in1=old[:],
                    op=mybir.AluOpType.subtract)
                e = gp_.tile([NPART, hh, G], F32, tag="e", name="e")
                nc.gpsimd.tensor_tensor(
                    out=e[:], in0=hg[:], in1=d[:],
                    op=mybir.AluOpType.mult)
                o = gp_.tile([NPART, hh, G], F32, tag="o", name="o")
                nc.vector.tensor_tensor(
                    out=o[:], in0=old[:], in1=e[:],
                    op=mybir.AluOpType.add)
                nc.sync.dma_start(
                    out_d[:, sl],
                    o[:].rearrange("p a b -> p (a b)"))

            def emit_old_dma(rnd, olds_out):
                for s in rnd:
                    if not s["last"]:
                        olds_out.append(None)
                        continue
                    z0, y0, hh = s["z0"], s["y0"], s["hh"]
                    sl = slice(z0 * 1024 + y0 * G, z0 * 1024 + (y0 + hh) * G)
                    old = gp_.tile([NPART, hh, G], F32, tag="old", name="old",
                                   bufs=2 * rounds + 2)
                    nc.sync.dma_start(
                        old[:], state_in[:, sl].rearrange(
                            "p (a b) -> p a b", a=hh))
                    olds_out.append(old)

            pend = None  # (round, ps1s, ys, olds)
            for rnd in round_list:
                ps1s = []
                emit_convs(rnd, ps1s)
                olds = []
                emit_old_dma(rnd, olds)
                ys = []
                for si, s in enumerate(rnd):
                    emit_tanh(s, ps1s[si], ys)
                if not pipeline:
                    ps2s = []
                    for si, s in enumerate(rnd):
                        emit_mix(s, ys[si], ps2s)
                    for si, s in enumerate(rnd):
                        emit_blend(s, ps2s[si], olds[si])
                    continue
                if pend is not None:
                    prnd, pys, polds = pend
                    ps2s = []
                    for si, s in enumerate(prnd):
                        emit_mix(s, pys[si], ps2s)
                    for si, s in enumerate(prnd):
                        emit_blend(s, ps2s[si], polds[si])
                pend = (rnd, ys, olds)
            if pipeline and pend is not None:
                prnd, pys, polds = pend
                ps2s = []
                for si, s in enumerate(prnd):
                    emit_mix(s, pys[si], ps2s)
                for si, s in enumerate(prnd):
                    emit_blend(s, ps2s[si], polds[si])

    nc.compile()
    return nc


_NC_CACHE = {}


def _get_nc(repeat_passes=1, active_taps=None, fold_taps=None,
            strided_rhs=True, pipeline=True, rounds=2):
    key = (repeat_passes, active_taps, fold_taps, strided_rhs, pipeline,
           rounds)
    if key not in _NC_CACHE:
        _NC_CACHE[key] = build_nc(repeat_passes, active_taps, fold_taps,
                                  strided_rhs, pipeline, rounds)
    return _NC_CACHE[key]


def make_in_maps(state, all_weights, all_biases, field_mix_logits,
                 pass_alpha_logit, gate_weight, gate_bias):
    convw8, mixw, pvec, active, fold = _host_params(
        all_weights, all_biases, field_mix_logits, pass_alpha_logit,
        gate_weight, gate_bias)
    state = np.ascontiguousarray(np.asarray(state, np.float32))
    in_maps = []
    for i in range(NCORES):
        shard = state[BL * i:BL * (i + 1)].reshape(NPART, G3)
        in_maps.append({
            "state_in": shard,
            "convw": convw8,
            "mixw": mixw,
            "pvec": pvec,
        })
    return in_maps, active, fold


def kernel(state, all_weights, all_biases, field_mix_logits,
           pass_alpha_logit, gate_weight, gate_bias):
    in_maps, active, fold = make_in_maps(state, all_weights, all_biases,
                                         field_mix_logits, pass_alpha_logit,
                                         gate_weight, gate_bias)
    nc = _get_nc(1, active, fold)
    for attempt in range(5):
        try:
            res = run_bass_kernel_spmd(nc, in_maps, core_ids=list(range(NCORES)))
            break
        except Exception:  # transient device-recovery errors
            if attempt == 4:
                raise
            import time as _time
            _time.sleep(10.0 * (attempt + 1))
    outs = [res.results[i]["out"].reshape(BL, F, C, G, G, G)
            for i in range(NCORES)]
    return np.concatenate(outs, axis=0).astype(np.float32)


# revision 27
# speedup vs baseline: 1.6316x; 1.6316x over previous
"""Trainium2 Bass kernel for nn_CliffordEngine (8-core data-parallel over batch).

Model: 4 passes of (grouped causal 3x3x3 conv over 32^3 grid -> tanh ->
8x8 cross-field mix -> alpha blend), then a sigmoid gate vs the original
state.  B=16, F=8 fields, 8 multivector components, G=32.

Mapping: each core takes 2 batch elements.  SBUF layout: partitions =
(b2, f8, c8) = 128; free dim = causally padded 34^3 volume stored in
fp8e4 (state scaled by S_X).  The conv runs as fp8 DoubleRow matmuls:
each instruction contracts TWO taps at once (K=256) using a strided
3D access pattern [128, 2(pair-delta), N] over the padded volume.

Scheduling: software-pipelined.  Conv matmuls for a round of `ROUNDS`
subtiles are emitted tap-major (consecutive matmuls share lhsT so the
PE can skip stationary reloads); the mix matmuls of the previous round
are emitted after the next round's convs so the PE never stalls on the
Act-engine tanh.  The final gate is computed with DVE+Pool ops split so
no single engine exceeds the PE's per-subtile budget.
"""

import numpy as np

import concourse.bacc as bacc
import concourse.tile as tile
from concourse import mybir
from concourse.ap import AP
from concourse.bass_utils import run_bass_kernel_spmd

# Cl(3,0) Cayley table, transcribed from the reference model.
_TABLE = [
    [(0, 0, 1), (1, 1, 1), (2, 2, 1), (3, 3, 1), (4, 4, -1), (5, 5, -1), (6, 6, -1), (7, 7, -1)],
    [(1, 0, 1), (0, 1, 1), (4, 2, -1), (5, 3, -1), (2, 4, 1), (3, 5, 1), (7, 6, -1), (6, 7, -1)],
    [(2, 0, 1), (4, 1, 1), (0, 2, 1), (6, 3, -1), (1, 4, -1), (7, 5, 1), (3, 6, 1), (5, 7, 1)],
    [(3, 0, 1), (5, 1, 1), (6, 2, 1), (0, 3, 1), (7, 4, -1), (1, 5, -1), (2, 6, -1), (4, 7, -1)],
    [(4, 0, 1), (2, 1, 1), (1, 2, -1), (7, 3, 1), (0, 4, 1), (6, 5, -1), (5, 6, 1), (3, 7, 1)],
    [(5, 0, 1), (3, 1, 1), (7, 2, -1), (1, 3, -1), (6, 4, 1), (0, 5, 1), (4, 6, -1), (2, 7, -1)],
    [(6, 0, 1), (7, 1, 1), (3, 2, 1), (2, 3, -1), (5, 4, -1), (4, 5, 1), (0, 6, 1), (1, 7, 1)],
    [(7, 0, 1), (6, 1, 1), (5, 2, -1), (4, 3, 1), (3, 4, 1), (2, 5, -1), (1, 6, 1), (0, 7, 1)],
]

B, F, C, G = 16, 8, 8, 32
P_PASSES = 4
NCORES = 8
BL = B // NCORES            # local batch per core = 2
NPART = BL * F * C          # 128
GP = G + 2                  # padded extent 34
G3 = G * G * G
NTAPS = 27
ROWST = GP                  # 34
PLANEST = GP * GP           # 1156
PITCH = GP * GP * GP + 128  # padded volume + tail slop for pair reads
S_X = 16.0                  # fp8 state scale
S_W = 128.0                 # fp8 conv weight scale
HEIGHTS = [(0, 16), (16, 16)]   # (y0, rows) subtiles per plane
TAPS = [(kd, kh, kw) for kd in range(3) for kh in range(3) for kw in range(3)]

F32 = mybir.dt.float32
BF16 = mybir.dt.bfloat16
FP8 = mybir.dt.float8e4
NP_FP8 = mybir.dt.np(FP8)
NP_BF16 = mybir.dt.np(BF16)


def _softmax(x, axis):
    m = np.max(x, axis=axis, keepdims=True)
    e = np.exp(x - m)
    return e / np.sum(e, axis=axis, keepdims=True)


def _host_params(all_weights, all_biases, field_mix_logits, pass_alpha_logit,
                 gate_weight, gate_bias):
    """Precompute device weight tensors (runtime data, not baked into the NEFF)."""
    T = np.zeros((8, 8, 8), np.float32)
    for i, row in enumerate(_TABLE):
        for j, k, s in row:
            T[i, j, k] = s
    aw = np.asarray(all_weights, np.float32)          # [F, P, 27, 8]
    # W_eff[f,p,i,j,t] : out component i, in component j, tap t
    W = np.einsum('ijk,fpck->fpijc', T, aw)            # [F,P,8,8,27]
    mix = _softmax(np.asarray(field_mix_logits, np.float32), axis=2)  # [P,F,F] (g,f)
    alpha = 1.0 / (1.0 + np.exp(-np.asarray(pass_alpha_logit, np.float32)))  # [P]

    # conv lhsT: [k=(b,f,j), p, t, m=(b,f,i)] block-diagonal over (b, f), fp8
    convw = np.zeros((BL, F, C, P_PASSES, NTAPS, BL, F, C), np.float32)
    A = np.transpose(W, (0, 3, 1, 4, 2))               # [f, j, p, t, i]
    for b in range(BL):
        for f in range(F):
            convw[b, f, :, :, :, b, f, :] = A[f]
    convw = convw.reshape(NPART, P_PASSES, NTAPS, NPART)
    convw8 = np.clip(convw * S_W, -240.0, 240.0).astype(NP_FP8)

    # mix lhsT: [k=(b,f,ci), p, m=(b,g,co)] = S_X (1-alpha_p) mix[p,g,f] delta_b delta_c
    mixw = np.zeros((BL, F, C, P_PASSES, BL, F, C), np.float32)
    for b in range(BL):
        for p in range(P_PASSES):
            M2 = S_X * (1.0 - alpha[p]) * mix[p].T     # [f, g]
            for ci in range(C):
                mixw[b, :, ci, p, b, :, ci] = M2
    mixw = mixw.reshape(NPART, P_PASSES, NPART).astype(NP_BF16)

    # per-partition scalars: cols 0-3 bias_p, 4-7 alpha_p, 8 = -gw, 9 = -gb,
    # 10 = alpha3/S_X, 11+p = S_W * diag of the folded center tap of pass p
    pvec = np.zeros((BL, F, C, 16), np.float32)
    pvec = pvec.reshape(NPART, 16)
    ab = np.asarray(all_biases, np.float32).reshape(F, P_PASSES, C)
    pv4 = pvec.reshape(BL, F, C, 16)
    for p in range(P_PASSES):
        pv4[:, :, :, p] = ab[None, :, p, :]
        pv4[:, :, :, 4 + p] = alpha[p]
    pv4[:, :, :, 8] = -np.asarray(gate_weight, np.float32)[None]
    pv4[:, :, :, 9] = -np.asarray(gate_bias, np.float32)[None]
    pv4[:, :, :, 10] = alpha[P_PASSES - 1] / S_X
    # last pass's mix output feeds the fused gate directly (f32, unscaled)
    mixw[:, P_PASSES - 1] = (mixw[:, P_PASSES - 1].astype(np.float32)
                             / S_X).astype(NP_BF16)

    # active taps per pass; fold one diagonal tap to DVE when the count is odd
    active, fold = [], []
    for p in range(P_PASSES):
        nz = [t for t in range(NTAPS) if np.any(convw[:, p, t, :] != 0)]
        if not nz:
            nz = [0]
        ft = None
        if len(nz) % 2 == 1 and len(nz) > 1:
            for t in reversed(nz):
                blk = convw[:, p, t, :]
                if not np.any(blk[~np.eye(NPART, dtype=bool)]):
                    ft = t
                    break
            if ft is not None:
                nz.remove(ft)
                pvec[:, 11 + p] = S_W * np.diag(convw[:, p, ft, :])
        active.append(tuple(nz))
        fold.append(ft)
    return convw8, mixw, pvec, tuple(active), tuple(fold)


def build_nc(repeat_passes=1, active_taps=None, fold_taps=None,
             strided_rhs=True, pipeline=True, rounds=2,
             probe_lhsT=False, probe_rhs=False, probe_nodr=False,
             probe_nomix=False, probe_pairs=None, bir_lowering=False,
             dedup_ldw=False, sparse_sem=False):
    """Build the per-core Bass program.

    pipeline: defer each round's mix matmuls until after the next round's
    conv matmuls so the PE never waits on the Act-engine tanh.
    rounds: subtiles per round; conv matmuls are emitted tap-major across
    the round so consecutive matmuls share the same stationary weights.
    probe_lhsT / probe_rhs: TIMING-ONLY probes (wrong numerics): share one
    stationary operand / use a contiguous moving operand for all conv
    matmuls, to isolate LD_WEIGHTS and rhs-access-pattern costs.
    """
    if active_taps is None:
        active_taps = tuple(tuple(range(NTAPS)) for _ in range(P_PASSES))
    if fold_taps is None:
        fold_taps = (None,) * P_PASSES
    nc = bacc.Bacc("TRN2", target_bir_lowering=bir_lowering, debug=False,
                   num_devices=NCORES)
    state_in = nc.dram_tensor("state_in", [NPART, G3], F32, kind="ExternalInput")
    convw_d = nc.dram_tensor("convw", [NPART, P_PASSES, NTAPS, NPART], FP8,
                             kind="ExternalInput")
    mixw_d = nc.dram_tensor("mixw", [NPART, P_PASSES, NPART], BF16,
                            kind="ExternalInput")
    pvec_d = nc.dram_tensor("pvec", [NPART, 16], F32, kind="ExternalInput")
    out_d = nc.dram_tensor("out", [NPART, G3], F32, kind="ExternalOutput")

    # Persistent padded state buffers (ping/pong), fp8, flat with tail slop.
    ping = nc.alloc_sbuf_tensor("ping", [NPART, PITCH], FP8)
    pong = nc.alloc_sbuf_tensor("pong", [NPART, PITCH], FP8)

    def vol_ap(vol, off, dims):
        return AP(vol, off, [[PITCH, NPART]] + dims)

    # per-subtile metadata: (pass, z0, y0, hh, cur, nxt, global index)
    subtiles = []
    vols = [ping, pong]
    for rp in range(repeat_passes):
        for p in range(P_PASSES):
            gp_idx = rp * P_PASSES + p
            cur, nxt = vols[gp_idx % 2], vols[(gp_idx + 1) % 2]
            last_pass = (rp == repeat_passes - 1 and p == P_PASSES - 1)
            for z0 in range(G):
                for (y0, hh) in HEIGHTS:
                    subtiles.append(dict(p=p, z0=z0, y0=y0, hh=hh, cur=cur,
                                         nxt=nxt, last=last_pass))
    # group into rounds (same pass within a round; pass lengths are
    # multiples of `rounds` when rounds divides 64)
    assert 64 % rounds == 0
    round_list = [subtiles[i:i + rounds] for i in range(0, len(subtiles),
                                                        rounds)]

    with tile.TileContext(nc) as tc:
        with (
            tc.tile_pool(name="const", bufs=1) as constp,
            tc.tile_pool(name="stage", bufs=3) as stagep,
            tc.tile_pool(name="ytile", bufs=2 * rounds + 1) as yp,
            tc.tile_pool(name="gtile", bufs=3) as gp_,
            tc.tile_pool(name="psum1", bufs=5, space="PSUM") as ps1p,
            tc.tile_pool(name="psum2", bufs=3, space="PSUM") as ps2p,
        ):
            # memset only the halo/pad regions (interior is fully overwritten
            # by the init load / blends).
            for vol in (ping, pong):
                nc.gpsimd.memset(vol[:, 0:2 * PLANEST + 2 * ROWST + 2], 0)
                nc.gpsimd.memset(
                    AP(vol, 3 * PLANEST, [[PITCH, NPART], [PLANEST, GP - 3],
                                          [1, 2 * ROWST + 2]]), 0)
                nc.gpsimd.memset(
                    AP(vol, 2 * PLANEST + 2 * ROWST,
                       [[PITCH, NPART], [PLANEST, GP - 2], [ROWST, G], [1, 2]]),
                    0)
                nc.gpsimd.memset(vol[:, GP * PLANEST:], 0)

            convw_sb = constp.tile([NPART, P_PASSES, NTAPS, NPART], FP8,
                                   tag="convw")
            for p in range(P_PASSES):  # split so pass 0 weights land first
                nc.sync.dma_start(convw_sb[:, p], convw_d[:, p])
            mixw_sb = constp.tile([NPART, P_PASSES, NPART], BF16, tag="mixw")
            nc.sync.dma_start(mixw_sb[:], mixw_d[:])
            pvec_sb = constp.tile([NPART, 16], F32, tag="pvec")
            nc.sync.dma_start(pvec_sb[:], pvec_d[:])

            # initial load: f32 planes -> fp8 (x S_X) padded interior
            for z in range(G):
                st = stagep.tile([NPART, G * G], F32, tag="stage")
                nc.sync.dma_start(st[:], state_in[:, z * 1024:(z + 1) * 1024])
                nc.scalar.activation(
                    out=vol_ap(ping, (z + 2) * PLANEST + 2 * ROWST + 2,
                               [[ROWST, G], [1, G]]),
                    in_=st[:].rearrange("p (a b) -> p a b", a=G),
                    func=mybir.ActivationFunctionType.Copy, scale=S_X)

            def sub_ap(base_ap, off, dims):
                return AP(base_ap.tensor, base_ap.offset + off,
                          [list(base_ap.ap[0])] + dims)

            def conv_lhsT(p, tA, tB):
                off = (p * NTAPS + tA) * NPART
                return sub_ap(convw_sb[:], off,
                              [[(tB - tA) * NPART, 2], [1, NPART]])

            def emit_convs(rnd, tiles_out):
                """Emit the conv matmuls for one round, tap-major."""
                p = rnd[0]["p"]
                act = active_taps[p]
                pairs = [(act[2 * i], act[2 * i + 1])
                         for i in range(len(act) // 2)]
                if probe_pairs is not None:
                    pairs = pairs[:probe_pairs]
                single = act[-1] if len(act) % 2 else None
                ngroups = len(pairs) + (1 if single is not None else 0)
                for s in rnd:
                    if strided_rhs:
                        ps1 = ps1p.tile([NPART, s["hh"], G], F32,
                                        space="PSUM", tag="ps1", name="ps1")
                    else:
                        ps1 = ps1p.tile([NPART, s["hh"] * ROWST], F32,
                                        space="PSUM", tag="ps1", name="ps1")
                    tiles_out.append(ps1)
                gi = 0
                for (tA, tB) in pairs:
                    if probe_lhsT:
                        tA, tB = pairs[0]
                    kdA, khA, kwA = TAPS[tA]
                    kdB, khB, kwB = TAPS[tB]
                    for si, s in enumerate(rnd):
                        offA = ((s["z0"] + kdA) * PLANEST
                                + (s["y0"] + khA) * ROWST + kwA)
                        offB = ((s["z0"] + kdB) * PLANEST
                                + (s["y0"] + khB) * ROWST + kwB)
                        if probe_nodr:
                            rhs = vol_ap(s["cur"], offA,
                                         [[ROWST, s["hh"]], [1, G]])
                            nc.tensor.matmul(
                                out=tiles_out[-len(rnd) + si][:],
                                lhsT=convw_sb[:, p, tA], rhs=rhs,
                                start=(gi == 0), stop=(gi == ngroups - 1))
                            continue
                        if probe_rhs:
                            rhs = vol_ap(s["cur"], offA,
                                         [[offB - offA, 2], [1, s["hh"] * G]])
                        elif strided_rhs:
                            rhs = vol_ap(s["cur"], offA,
                                         [[offB - offA, 2],
                                          [ROWST, s["hh"]], [1, G]])
                        else:
                            rhs = vol_ap(s["cur"], offA,
                                         [[offB - offA, 2],
                                          [1, s["hh"] * ROWST]])
                        inst = nc.tensor.matmul(
                            out=tiles_out[-len(rnd) + si][:],
                            lhsT=conv_lhsT(p, tA, tB),
                            rhs=rhs, start=(gi == 0),
                            stop=(gi == ngroups - 1),
                            perf_mode=mybir.MatmulPerfMode.DoubleRow)
                        if dedup_ldw and si > 0:
                            inst.ldweights = False
                    gi += 1
                if single is not None:
                    kd, kh, kw = TAPS[single]
                    for si, s in enumerate(rnd):
                        off = ((s["z0"] + kd) * PLANEST
                               + (s["y0"] + kh) * ROWST + kw)
                        if strided_rhs:
                            rhs = vol_ap(s["cur"], off,
                                         [[ROWST, s["hh"]], [1, G]])
                        else:
                            rhs = vol_ap(s["cur"], off,
                                         [[1, s["hh"] * ROWST]])
                        inst = nc.tensor.matmul(
                            out=tiles_out[-len(rnd) + si][:],
                            lhsT=convw_sb[:, p, single], rhs=rhs,
                            start=(gi == 0), stop=(gi == ngroups - 1))
                        if dedup_ldw and si > 0:
                            inst.ldweights = False
                    gi += 1

            def emit_tanh(s, ps1, ys_out):
                """Act tanh (+ optional DVE fold of a diagonal center tap)."""
                p, hh = s["p"], s["hh"]
                if strided_rhs:
                    ps1v = ps1[:]
                else:
                    ps1v = sub_ap(ps1[:], 0, [[ROWST, hh], [1, G]])
                tanh_in = ps1v
                if fold_taps[p] is not None:
                    kd, kh, kw = TAPS[fold_taps[p]]
                    coff = ((s["z0"] + kd) * PLANEST
                            + (s["y0"] + kh) * ROWST + kw)
                    tmp = yp.tile([NPART, hh, G], F32, tag="tmp", name="tmp")
                    nc.vector.scalar_tensor_tensor(
                        out=tmp[:],
                        in0=vol_ap(s["cur"], coff, [[ROWST, hh], [1, G]]),
                        scalar=pvec_sb[:, 11 + p:12 + p],
                        in1=ps1v,
                        op0=mybir.AluOpType.mult,
                        op1=mybir.AluOpType.add,
                    )
                    tanh_in = tmp[:]
                y = yp.tile([NPART, hh, G], BF16, tag="y", name="y")
                nc.scalar.activation(
                    out=y[:], in_=tanh_in,
                    func=mybir.ActivationFunctionType.Tanh,
                    bias=pvec_sb[:, p:p + 1], scale=1.0 / (S_X * S_W),
                )
                ys_out.append(y)

            def emit_mix(s, y, ps2_out, reuse_w=False):
                p, hh = s["p"], s["hh"]
                ps2 = ps2p.tile([NPART, hh, G], F32, space="PSUM", tag="ps2",
                                name="ps2")
                inst = nc.tensor.matmul(out=ps2[:], lhsT=mixw_sb[:, p],
                                        rhs=y[:], start=True, stop=True)
                if dedup_ldw and reuse_w:
                    inst.ldweights = False
                ps2_out.append(ps2)

            def emit_blend(s, ps2, old=None):
                p, hh = s["p"], s["hh"]
                z0, y0 = s["z0"], s["y0"]
                intr = (z0 + 2) * PLANEST + (y0 + 2) * ROWST + 2
                if not s["last"]:
                    # nxt = alpha * cur + ps2  (S_X units, fp8)
                    nc.vector.scalar_tensor_tensor(
                        out=vol_ap(s["nxt"], intr, [[ROWST, hh], [1, G]]),
                        in0=vol_ap(s["cur"], intr, [[ROWST, hh], [1, G]]),
                        scalar=pvec_sb[:, 4 + p:5 + p],
                        in1=ps2[:],
                        op0=mybir.AluOpType.mult,
                        op1=mybir.AluOpType.add,
                    )
                    return
                # fused gate (last pass): ps2 here is unscaled, so
                # x4 = (alpha3/S_X) * cur + ps2;
                # out = old + sigmoid(-(gw*old+gb)) * (x4 - old)
                sl = slice(z0 * 1024 + y0 * G, z0 * 1024 + (y0 + hh) * G)
                hg = gp_.tile([NPART, hh, G], F32, tag="hg", name="hg")
                nc.scalar.activation(
                    out=hg[:], in_=old[:],
                    func=mybir.ActivationFunctionType.Sigmoid,
                    bias=pvec_sb[:, 9:10], scale=pvec_sb[:, 8:9],
                )
                x4 = gp_.tile([NPART, hh, G], F32, tag="x4", name="x4")
                nc.vector.scalar_tensor_tensor(
                    out=x4[:],
                    in0=vol_ap(s["cur"], intr, [[ROWST, hh], [1, G]]),
                    scalar=pvec_sb[:, 10:11],
                    in1=ps2[:],
                    op0=mybir.AluOpType.mult,
                    op1=mybir.AluOpType.add,
                )
                d = gp_.tile([NPART, hh, G], F32, tag="d", name="d")
                nc.gpsimd.tensor_tensor(
                    out=d[:], in0=x4[:], in1=old[:],
                    op=mybir.AluOpType.subtract)
                e = gp_.tile([NPART, hh, G], F32, tag="e", name="e")
                nc.gpsimd.tensor_tensor(
                    out=e[:], in0=hg[:], in1=d[:],
                    op=mybir.AluOpType.mult)
                o = gp_.tile([NPART, hh, G], F32, tag="o", name="o")
                nc.vector.tensor_tensor(
                    out=o[:], in0=old[:], in1=e[:],
                    op=mybir.AluOpType.add)
                nc.sync.dma_start(
                    out_d[:, sl],
                    o[:].rearrange("p a b -> p (a b)"))

            def emit_old_dma(rnd, olds_out):
                for s in rnd:
                    if not s["last"]:
                        olds_out.append(None)
                        continue
                    z0, y0, hh = s["z0"], s["y0"], s["hh"]
                    sl = slice(z0 * 1024 + y0 * G, z0 * 1024 + (y0 + hh) * G)
                    old = gp_.tile([NPART, hh, G], F32, tag="old", name="old",
                                   bufs=2 * rounds + 2)
                    nc.sync.dma_start(
                        old[:], state_in[:, sl].rearrange(
                            "p (a b) -> p a b", a=hh))
                    olds_out.append(old)

            pend = None  # (round, ps1s, ys, olds)
            for rnd in round_list:
                ps1s = []
                emit_convs(rnd, ps1s)
                olds = []
                emit_old_dma(rnd, olds)
                ys = []
                for si, s in enumerate(rnd):
                    emit_tanh(s, ps1s[si], ys)
                def emit_post(prnd, pys, polds):
                    if probe_nomix:
                        for si, s in enumerate(prnd):
                            emit_blend(s, pys[si], polds[si])
                        return
                    ps2s = []
                    for si, s in enumerate(prnd):
                        emit_mix(s, pys[si], ps2s, reuse_w=(si > 0))
                    for si, s in enumerate(prnd):
                        emit_blend(s, ps2s[si], polds[si])

                if not pipeline:
                    emit_post(rnd, ys, olds)
                    continue
                if pend is not None:
                    emit_post(*pend)
                pend = (rnd, ys, olds)
            if pipeline and pend is not None:
                emit_post(*pend)

    nc.compile()
    if dedup_ldw:
        _dedup_ldweights(nc)
    if sparse_sem:
        _sparsify_matmul_sems(nc)
    return nc


def _sparsify_matmul_sems(nc):
    """Matmuls inside a PSUM accumulation group post a semaphore increment
    each; downstream waits only ever need group completion.  Strip the
    increments from non-final (stop_tensor_calc=False) matmuls and remap
    every wait threshold on the affected semaphores to the next kept
    increment (engines complete in order, so waiting for a later increment
    is always sufficient)."""
    import bisect

    for fn in nc.m.functions:
        insts = [i for b in fn.blocks for i in b.instructions]
        # gather per-sem update streams
        upd_stream = {}  # sem_id -> list of (seq, inst, is_strippable)
        bail = set()
        for seq, i in enumerate(insts):
            si = i.sync_info
            if si is None:
                continue
            for u in (si.on_update or []):
                if u.update_mode != "sem-inc" or u.update_value != 1 or \
                        u.update_reg is not None:
                    bail.add(u.id)
                    continue
                strippable = (type(i).__name__ == "InstMatmult"
                              and i.stop_tensor_calc is False)
                upd_stream.setdefault(u.id, []).append((seq, i, strippable))
            for w in (si.on_wait or []):
                if w.wait_mode != "sem-ge-imm" or w.wait_reg is not None:
                    bail.add(w.id)
        for sem_id, stream in upd_stream.items():
            if sem_id in bail:
                continue
            if not any(s for _, _, s in stream):
                continue
            # old cumulative k (1-based) -> new threshold:
            # number of kept updates up to and including the first kept
            # update at or after position k.
            kept_flags = [not s for _, _, s in stream]
            n = len(stream)
            new_thresh = [0] * (n + 1)
            kept_cum = 0
            # next_kept_newcount[k]: new threshold covering old count k
            # walk backwards to find next kept index
            kept_cum_arr = []
            c = 0
            for f in kept_flags:
                if f:
                    c += 1
                kept_cum_arr.append(c)
            total_kept = c
            nxt = [0] * (n + 1)
            nk = total_kept + 1  # sentinel (no later kept update)
            for k in range(n, 0, -1):
                if kept_flags[k - 1]:
                    nk = kept_cum_arr[k - 1]
                nxt[k] = nk
            ok = True
            for k in range(1, n + 1):
                if nxt[k] > total_kept:
                    ok = False  # stripped update with no later kept one
                    break
                new_thresh[k] = nxt[k]
            if not ok:
                continue
            # rewrite waits
            for i in insts:
                si = i.sync_info
                if si is None:
                    continue
                changed = False
                for w in (si.on_wait or []):
                    if w.id == sem_id:
                        v = w.wait_value
                        if 1 <= v <= n:
                            w.wait_value = new_thresh[v]
                            changed = True
                        elif v > n:
                            w.wait_value = total_kept
                            changed = True
                if changed:
                    i.sync_info = si
            # strip updates
            for seq, i, s in stream:
                if not s:
                    continue
                si = i.sync_info
                si.on_update = [u for u in si.on_update if u.id != sem_id]
                i.sync_info = si


def _dedup_ldweights(nc):
    """Delete redundant InstLdweights: within a basic block, an Ldweights
    whose weights AP/perf-mode/tile config exactly matches the previous
    PE weight load (with no other weight-affecting PE instruction between)
    is a no-op for the PE array.  All Ldweights here carry no sync info
    (waits/updates live on matmuls and event semaphores), so deletion
    preserves the synchronization structure."""
    removed = 0
    for fn in nc.m.functions:
        for b in fn.blocks:
            insts = list(b.instructions)
            out = []
            last_key = None
            for i in insts:
                t = type(i).__name__
                if t == "InstLdweights":
                    si = i.sync_info
                    has_sync = si is not None and (
                        getattr(si, "on_wait", None) or
                        getattr(si, "on_update", None))
                    key = (str(i.ins[0]), str(i.perf_mode),
                           str(i.tile_position), str(i.tile_size),
                           str(i.is_transpose))
                    if not has_sync and key == last_key:
                        removed += 1
                        continue
                    last_key = key
                elif t == "InstMatmult":
                    if getattr(i, "is_transpose", False):
                        last_key = None
                elif t in ("InstEventSemaphore", "InstNop"):
                    pass  # no effect on PE weight state
                else:
                    # conservatively assume anything else on the PE stream
                    # could clobber weights
                    if getattr(i, "engine", None) == mybir.EngineType.PE:
                        last_key = None
                out.append(i)
            if removed and len(out) != len(insts):
                b.instructions = out
    return removed


_NC_CACHE = {}


def _get_nc(repeat_passes=1, active_taps=None, fold_taps=None,
            strided_rhs=True, pipeline=True, rounds=2, **probes):
    key = (repeat_passes, active_taps, fold_taps, strided_rhs, pipeline,
           rounds) + tuple(sorted(probes.items()))
    if key not in _NC_CACHE:
        _NC_CACHE[key] = build_nc(repeat_passes, active_taps, fold_taps,
                                  strided_rhs, pipeline, rounds, **probes)
    return _NC_CACHE[key]


def make_in_maps(state, all_weights, all_biases, field_mix_logits,
                 pass_alpha_logit, gate_weight, gate_bias):
    convw8, mixw, pvec, active, fold = _host_params(
        all_weights, all_biases, field_mix_logits, pass_alpha_logit,
        gate_weight, gate_bias)
    state = np.ascontiguousarray(np.asarray(state, np.float32))
    in_maps = []
    for i in range(NCORES):
        shard = state[BL * i:BL * (i + 1)].reshape(NPART, G3)
        in_maps.append({
            "state_in": shard,
            "convw": convw8,
            "mixw": mixw,
            "pvec": pvec,
        })
    return in_maps, active, fold


def kernel(state, all_weights, all_biases, field_mix_logits,
           pass_alpha_logit, gate_weight, gate_bias):
    in_maps, active, fold = make_in_maps(state, all_weights, all_biases,
                                         field_mix_logits, pass_alpha_logit,
                                         gate_weight, gate_bias)
    nc = _get_nc(1, active, fold, True, False, 1)
    for attempt in range(5):
        try:
            res = run_bass_kernel_spmd(nc, in_maps, core_ids=list(range(NCORES)))
            break
        except Exception:  # transient device-recovery errors
            if attempt == 4:
                raise
            import time as _time
            _time.sleep(10.0 * (attempt + 1))
    outs = [res.results[i]["out"].reshape(BL, F, C, G, G, G)
            for i in range(NCORES)]
    return np.concatenate(outs, axis=0).astype(np.float32)
